# revision 42
# baseline (speedup 1.0000x reference)
"""Grouped-query causal attention on 8 TRN2 NeuronCores.

Problem: q [B=2, S=2048, H=32, D=128], k/v [B=2, S=2048, HKV=8, D=128],
causal softmax(q k^T / sqrt(D)) v with G = H // HKV = 4 query heads per
kv head.

Sharding (no collectives needed): 8 cores = 2 batches x 4 kv-head-pairs.
Each core computes 8 query heads / 2 kv heads of one batch element.

Per-core kernel design:
  - scores are built TRANSPOSED (S^T[k, q] tiles, k on partitions) so that
    softmax(P^T) feeds the P@V matmul directly as lhsT with no on-chip
    transposes at all.
  - Q^T/K^T [d, s] layouts come from a bf16 DRAM bounce (gpsimd casting
    DMA fp32->bf16) followed by a 2-byte xbar DMA transpose load.
  - causality is exploited at the slab level: per (q-block, k-tile-group)
    score slabs of [128, 1536] (3 PSUM banks, KG=3 k-tiles) for the fully
    causal region, plus ONE packed diagonal slab per q-block holding only
    the causal-needed columns of the 4 diagonal k-tiles (512+384+256+128 =
    1280 of 2048 cols; segments are packed to never cross a PSUM bank).
    This cuts both QK matmul columns and exp work by ~15% vs full-width.
  - exp is split across two engines to break the ScalarE bottleneck:
      * full slabs + qb0's diagonal slab: ScalarE ACTIVATE exp (exact),
        [128, up-to-1536] per instruction (amortizes the ~350cyc/instr
        overhead), bf16 out straight to SBUF.
      * diagonal slabs of qb1..3: VectorE via a Schraudolph bf16 exp:
        ONE fused scalar_tensor_tensor op computes
          int16 out = round(score * (128*scale/ln2) + maskbias)
        where maskbias = 16250.5 normally and +1200 on the masked upper
        triangle of diagonal blocks; the int16 bit pattern reinterpreted
        as bf16 IS 2^(t) with a linear-mantissa approx (~2% rel err on
        ~22% of elements -> ~0.5% output rel err), and masked entries
        become 2^-109 ~= 0. exp+causal-mask in one DVE op, no ScalarE.
  - softmax denominators ride along the P@V matmul as a ones-column
    appended to V (output column 128 = row sums), so no reductions are
    needed anywhere; finalize is a batched strided reciprocal + ONE
    broadcast tensor_tensor multiply per acc tile (2 q-tiles at a time).
  - PSUM budget: 2 score slabs (3 banks each) double-buffered + 2 acc
    tiles (1 bank each, 2 accumulator regions packed per bank) = 8 banks.
"""

import numpy as np

_B, _S, _H, _HKV, _D = 2, 2048, 32, 8, 128
_G = _H // _HKV  # 4 query heads per kv head
_NCORES = 8
_SHARDS = 4  # head shards; cores = _B * _SHARDS
_H_PER = _H // _SHARDS  # 8
_KV_PER = _HKV // _SHARDS  # 2

_P = 128  # partition / tile edge
_QB = 512  # q columns per block (4 q tiles)
_KG = 3  # k tiles per full PSUM score slab (3 banks)

# packed diagonal slab layout: local k-tile l -> (col offset, width)
# widths 512,384,256,128; 384+128 share a bank (384@512, 128@896) so no
# matmul output ever crosses a PSUM bank boundary.
_DIAG_OFF = {0: 0, 1: 512, 2: 1024, 3: 896}
_DIAG_W = {0: 512, 1: 384, 2: 256, 3: 128}
_DIAG_TOTAL = 1280

# Schraudolph bf16 exp constants: int16 = round(score*_SCHR_A + _SCHR_B)
_SCHR_A = float(_P / np.log(2.0) * (_D ** -0.5))  # fold 1/sqrt(D) scaling in
_SCHR_B = float(16256.0 - 5.5)
_SCHR_MASK = 1200.0  # masked entries -> bits ~<2700 -> bf16 ~2^-106 ~= 0

_build_cache = {}


def build_program(S=_S, n_heads=_H_PER, n_kv=_KV_PER, g=_G):
    """Emit + compile the single-core Tile program (SPMD: same NEFF on all
    cores, only the input data differs)."""
    import concourse.mybir as mybir
    import concourse.tile as tile
    from concourse import bacc
    from concourse.tile import add_dep_helper
    from contextlib import ExitStack

    dt = mybir.dt
    AF = mybir.ActivationFunctionType
    ALU = mybir.AluOpType

    D, P, QB, KG = _D, _P, _QB, _KG
    n_qt = S // P  # 128-row tiles along the sequence
    n_qb = S // QB  # q blocks
    qtb = QB // P  # q tiles per block (4)
    scale = float(D) ** -0.5

    nc = bacc.Bacc("TRN2", target_bir_lowering=False, debug=False)
    q_in = nc.dram_tensor("q", [S, n_heads, D], dt.float32, kind="ExternalInput").ap()
    k_in = nc.dram_tensor("k", [S, n_kv, D], dt.float32, kind="ExternalInput").ap()
    v_in = nc.dram_tensor("v", [S, n_kv, D], dt.float32, kind="ExternalInput").ap()
    o_out = nc.dram_tensor("out", [S, n_heads, D], dt.float32, kind="ExternalOutput").ap()

    with tile.TileContext(nc) as tc, ExitStack() as ctx:
        const_pool = ctx.enter_context(tc.tile_pool(name="const", bufs=1))
        dram_pool = ctx.enter_context(tc.tile_pool(name="bounce", bufs=12, space="DRAM"))
        qt_pool = ctx.enter_context(tc.tile_pool(name="qT", bufs=6))
        kt_pool = ctx.enter_context(tc.tile_pool(name="kT", bufs=2))
        v_pool = ctx.enter_context(tc.tile_pool(name="vv", bufs=2))
        fstg_pool = ctx.enter_context(tc.tile_pool(name="fstg", bufs=4))
        pt_pool = ctx.enter_context(tc.tile_pool(name="pT", bufs=4))
        osb_pool = ctx.enter_context(tc.tile_pool(name="osb", bufs=4))
        rc_pool = ctx.enter_context(tc.tile_pool(name="rc", bufs=8))
        sc_pool = ctx.enter_context(tc.tile_pool(name="sc", bufs=2, space="PSUM"))
        acc_pool = ctx.enter_context(tc.tile_pool(name="acc", bufs=2, space="PSUM"))

        def make_consts():
            # 0/1 lower-triangle (keep k<=q) for masking qb0's diagonal
            # blocks after its exact ScalarE exp.
            tri01 = const_pool.tile([P, P], dt.bfloat16)
            nc.gpsimd.memset(tri01[:], 1.0)
            nc.gpsimd.affine_select(
                out=tri01[:],
                in_=tri01[:],
                pattern=[[1, P]],
                base=0,
                channel_multiplier=-1,
                compare_op=ALU.is_ge,
                fill=0.0,
            )
            # Schraudolph bias-with-mask for the packed diagonal slab: the
            # value _SCHR_B everywhere, except the strict upper triangle
            # (k>q) of each segment's leading diagonal block, where a small
            # bias maps exp->~0.
            maskbias = const_pool.tile([P, _DIAG_TOTAL], dt.float32)
            nc.gpsimd.memset(maskbias[:], _SCHR_B)
            for l in range(qtb):
                off = _DIAG_OFF[l]
                nc.gpsimd.affine_select(
                    out=maskbias[:, off : off + P],
                    in_=maskbias[:, off : off + P],
                    pattern=[[1, P]],
                    base=0,
                    channel_multiplier=-1,
                    compare_op=ALU.is_ge,
                    fill=_SCHR_MASK,
                )
            return tri01, maskbias

        def load_xT_swdge(src, pool, tag, chunks=1):
            """SWDGE casting DMA(s) into a bf16 bounce + xbar load(s):
            slow (~40GB/s) but fully parallel to the HWDGE queues --
            carries the well-prefetched back-half tensors."""
            bounce = dram_pool.tile([S, D], dt.bfloat16, tag="bounce", name="bounce")
            xT = pool.tile([P, S], dt.bfloat16, tag=tag, name=tag)
            rows = S // chunks
            for c in range(chunks):
                sl = slice(c * rows, (c + 1) * rows)
                nc.gpsimd.dma_start(out=bounce[sl, :], in_=src[sl, :])
                nc.sync.dma_start_transpose(xT[:, sl], bounce[sl, :])
            return xT

        def cast_chunk_xT(src, xT, c, rows, eng, dq):
            """One chunk of the compute-cast transpose-load chain:
            fstg load -> compute cast (DVE 2x / gpsimd) -> bounce write ->
            xbar transpose load. All DMA hops of one chain ride ONE HWDGE
            queue (dq); different tensors' chains on different queues
            progress in parallel despite FIFO head-of-line blocking."""
            rt = rows // P
            sl = src[c * rows : (c + 1) * rows, :]
            stg = fstg_pool.tile([P, rt, D], dt.float32, tag="fstg", name="fstg")
            dq.dma_start(out=stg[:], in_=sl.rearrange("(t p) d -> p t d", p=P))
            stgb = fstg_pool.tile([P, rt, D], dt.bfloat16, tag="fstgb", name="fstgb")
            eng.tensor_copy(out=stgb[:], in_=stg[:])
            bounce = dram_pool.tile([rows, D], dt.bfloat16, tag="bounce", name="bounce")
            dq.dma_start(out=bounce[:].rearrange("(t p) d -> p t d", p=P), in_=stgb[:])
            dq.dma_start_transpose(xT[:, c * rows : (c + 1) * rows], bounce[:])

        def load_xT_cast(src, pool, tag, eng, chunks=1, dq=None):
            xT = pool.tile([P, S], dt.bfloat16, tag=tag, name=tag)
            rows = S // chunks
            for c in range(chunks):
                cast_chunk_xT(src, xT, c, rows, eng, dq or nc.sync)
            return xT

        def load_k(kv):
            if kv == 0:
                return load_xT_cast(k_in[:, kv, :], kt_pool, "kT", nc.vector, chunks=4)
            return load_xT_swdge(k_in[:, kv, :], kt_pool, "kT")

        def load_q(h):
            if h == 1:
                # startup-critical: rides the Scalar HWDGE queue, parallel
                # to head 0's chains on Sync
                return load_xT_cast(q_in[:, h, :], qt_pool, "qT", nc.vector,
                                    chunks=2, dq=nc.scalar)
            if h == 2:
                return load_xT_cast(q_in[:, h, :], qt_pool, "qT", nc.vector, chunks=2)
            if h == 3:
                return load_xT_cast(q_in[:, h, :], qt_pool, "qT", nc.gpsimd)
            return load_xT_swdge(q_in[:, h, :], qt_pool, "qT")

        def load_v_chunk(vv, kv, c, rt, eng):
            vstg = fstg_pool.tile([P, rt, D], dt.float32, tag="fstg", name="fstg")
            nc.sync.dma_start(
                out=vstg[:],
                in_=v_in[c * rt * P : (c + 1) * rt * P, kv, :].rearrange(
                    "(t p) d -> p t d", p=P
                ),
            )
            eng.tensor_copy(out=vv[:, c * rt : (c + 1) * rt, 0:D], in_=vstg[:])

        def load_v(kv):
            # fp32 strided load -> compute cast into the vv tile. Row pitch
            # D+2 keeps bf16 rows 4B-aligned so the DVE cast runs in 2x mode.
            vv = v_pool.tile([P, n_qt, D + 2], dt.bfloat16, tag="vv", name="vv")
            nc.gpsimd.memset(vv[:, :, D], 1.0)
            eng = nc.vector if kv == 0 else nc.gpsimd
            chunks = 4 if kv == 0 else 2
            rt = n_qt // chunks
            for c in range(chunks):
                load_v_chunk(vv, kv, c, rt, eng)
            return vv

        # prefetched tiles, keyed by head / kv-head index
        kTs, qTs, vvs = {}, {}, {}

        def prefetch(hh):
            if hh >= n_heads:
                return
            hkv = hh // g
            if hkv not in kTs:
                kTs[hkv] = load_k(hkv)
            if hh not in qTs:
                qTs[hh] = load_q(hh)
            if hkv not in vvs:
                vvs[hkv] = load_v(hkv)

        def prefetch0():
            """Head 0's loads: k chain on the Sync HWDGE queue and q chain
            on the Scalar HWDGE queue IN PARALLEL (each chain is internally
            FIFO-serialized; two queues let k c0 and q c0 both land within
            ~10us). v rides Sync fstg + DVE cast (no transpose needed)."""
            kT = kt_pool.tile([P, S], dt.bfloat16, tag="kT", name="kT")
            qT = qt_pool.tile([P, S], dt.bfloat16, tag="qT", name="qT")
            vv = v_pool.tile([P, n_qt, D + 2], dt.bfloat16, tag="vv", name="vv")
            nc.gpsimd.memset(vv[:, :, D], 1.0)
            rows = S // 4
            for c in range(4):
                cast_chunk_xT(k_in[:, 0, :], kT, c, rows, nc.vector, nc.sync)
                cast_chunk_xT(q_in[:, 0, :], qT, c, rows, nc.vector, nc.scalar)
                load_v_chunk(vv, 0, c, rows // P, nc.vector)
            kTs[0], qTs[0], vvs[0] = kT, qT, vv

        prefetch0()
        prefetch(1)
        tri01, maskbias = make_consts()
        prefetch(2)

        # Out-stores are deferred ~2 slabs so that, by the time the Sync
        # FIFO queue head reaches a store trigger, its finalize data is
        # already computed -- a store that waits at queue head blocks every
        # load DMA queued behind it.
        pending_stores = []

        def flush_stores(force=False):
            while pending_stores and (force or len(pending_stores) > 2):
                o_sb_p, qb_p, h_p = pending_stores.pop(0)
                nc.gpsimd.dma_start(
                    out=o_out[qb_p * QB : (qb_p + 1) * QB, h_p, :].rearrange(
                        "(t p) d -> p t d", p=P
                    ),
                    in_=o_sb_p[:],
                )

        # ------- global cross-head slab stream -------
        # The PE is in-order: if head h+1's first QK were only emitted
        # after head h's last PV, the PE would idle for the final diag
        # slab's DVE-exp latency at every head boundary. Building one
        # global slab stream lets the QK lookahead cross head boundaries.
        entries = []  # (qk_thunk, pv_thunk) per slab, all heads
        next_build = [0]

        def build_head(h):
            kv = h // g
            prefetch(h + 3)  # keep load chains ~3 head-windows ahead
            kT = kTs[kv]
            qT = qTs.pop(h)
            vv = vvs[kv]
            if h % g == g - 1:  # last head using this kv group
                del kTs[kv], vvs[kv]

            # slab list for this head:
            #   ("full", qb, j0, j1): k tiles [j0, j1) at full 512 width
            #   ("diag", qb): the 4 diagonal k-tiles, packed causal widths
            slabs = []
            for qb in range(n_qb):
                nfull = qb * qtb
                for j0 in range(0, nfull, KG):
                    slabs.append(("full", qb, j0, min(j0 + KG, nfull)))
                slabs.append(("diag", qb, 0, 0))

            accs_of = {}  # qb -> 2 accumulator tiles (2 regions each)
            live = {}  # slab idx -> pT tile

            def emit_qk(si, kT=kT, qT=qT, h=h, slabs=slabs, live=live):
                kind, qb, j0, j1 = slabs[si]
                sc = sc_pool.tile([P, KG * QB], dt.float32, tag="sc", name="sc")
                pT = pt_pool.tile([P, KG * QB], dt.bfloat16, tag="pT", name="pT")
                if kind == "full":
                    for j in range(j0, j1):
                        jl = j - j0
                        nc.tensor.matmul(
                            out=sc[:, jl * QB : (jl + 1) * QB],
                            lhsT=kT[:, j * P : (j + 1) * P],
                            rhs=qT[:, qb * QB : (qb + 1) * QB],
                            start=True,
                            stop=True,
                        )
                    W = (j1 - j0) * QB
                    nc.scalar.activation(
                        out=pT[:, :W], in_=sc[:, :W], func=AF.Exp, scale=scale
                    )
                else:
                    for l in range(qtb):
                        j = qb * qtb + l
                        off, w = _DIAG_OFF[l], _DIAG_W[l]
                        nc.tensor.matmul(
                            out=sc[:, off : off + w],
                            lhsT=kT[:, j * P : (j + 1) * P],
                            rhs=qT[:, qb * QB + l * P : (qb + 1) * QB],
                            start=True,
                            stop=True,
                        )
                    if qb == 0:
                        # exact exp; mask diag blocks after (VectorE, cheap)
                        nc.scalar.activation(
                            out=pT[:, :_DIAG_TOTAL],
                            in_=sc[:, :_DIAG_TOTAL],
                            func=AF.Exp,
                            scale=scale,
                        )
                        for l in range(qtb):
                            off = _DIAG_OFF[l]
                            blk = pT[:, off : off + P]
                            nc.vector.tensor_tensor(
                                out=blk, in0=blk, in1=tri01[:], op=ALU.mult
                            )
                    else:
                        # Schraudolph bf16 exp + causal mask, ONE DVE op:
                        # int16 = round(score * A + maskbias)
                        nc.vector.scalar_tensor_tensor(
                            out=pT[:, :_DIAG_TOTAL].bitcast(dt.int16),
                            in0=sc[:, :_DIAG_TOTAL],
                            scalar=_SCHR_A,
                            in1=maskbias[:],
                            op0=ALU.mult,
                            op1=ALU.add,
                        )
                live[si] = pT

            def emit_pv(si, vv=vv, h=h, slabs=slabs, live=live, accs_of=accs_of):
                kind, qb, j0, j1 = slabs[si]
                pT = live.pop(si)
                if (kind == "full" and j0 == 0) or (kind == "diag" and qb == 0):
                    # two accumulators packed per PSUM bank; region r of a
                    # tile is cols [r*(D+1), (r+1)*(D+1)). Only region 0's
                    # first matmul uses start=True (clears the whole bank's
                    # has_written bits); region 1's first matmul relies on
                    # still-pending bits to overwrite, so it must execute
                    # after region 0's start (manual dep below).
                    accs_of[qb] = [
                        acc_pool.tile([P, 2 * (D + 1)], dt.float32, tag="acc", name=f"accp{r}")
                        for r in range(qtb // 2)
                    ]
                accs = accs_of[qb]
                first_mm = {}

                def pv_mm(lhs_ap, it, first, last):
                    tile_, r = accs[it // 2], it % 2
                    mm = nc.tensor.matmul(
                        out=tile_[:, r * (D + 1) : (r + 1) * (D + 1)],
                        lhsT=lhs_ap,
                        rhs=vv[:, pv_mm.j, 0 : D + 1],
                        start=(first and r == 0),
                        stop=last,
                        skip_group_check=True,
                    )
                    if first:
                        first_mm[it] = mm
                        if r == 1:
                            add_dep_helper(
                                mm.ins,
                                first_mm[it - 1].ins,
                                sync=False,
                                reason="acc bank-mate ordering (pending-zero)",
                            )

                if kind == "full":
                    for j in range(j0, j1):
                        jl = j - j0
                        pv_mm.j = j
                        for it in range(qtb):
                            qt_abs = qb * qtb + it
                            pv_mm(
                                pT[:, jl * QB + it * P : jl * QB + (it + 1) * P],
                                it,
                                first=(j == 0),
                                last=False,
                            )
                else:
                    for l in range(qtb):
                        j = qb * qtb + l
                        off = _DIAG_OFF[l]
                        pv_mm.j = j
                        for it in range(l, qtb):
                            pv_mm(
                                pT[:, off + (it - l) * P : off + (it - l + 1) * P],
                                it,
                                first=(qb == 0 and l == 0),
                                last=(l == it),
                            )
                    # finalize this q block: batched reciprocal of the two
                    # ride-along denominators per acc tile, then ONE
                    # broadcast multiply per acc tile (2 q-tiles at once).
                    o_sb = osb_pool.tile([P, qtb, D], dt.float32, tag="osb", name="osb")
                    for t in range(qtb // 2):
                        tile_ = accs[t]
                        rc = rc_pool.tile([P, 2], dt.float32, tag="rc", name="rc")
                        nc.vector.reciprocal(rc[:], tile_[:, D :: D + 1])
                        nc.vector.tensor_tensor(
                            out=o_sb[:, 2 * t : 2 * t + 2, :],
                            in0=tile_[:].rearrange("p (t c) -> p t c", t=2)[:, :, 0:D],
                            in1=rc[:].unsqueeze(2).broadcast_to([P, 2, D]),
                            op=ALU.mult,
                        )
                    pending_stores.append((o_sb, qb, h))
                    del accs_of[qb]

            for si in range(len(slabs)):
                entries.append((lambda si=si, f=emit_qk: f(si),
                                lambda si=si, f=emit_pv: f(si)))

        def ensure_built(upto):
            while len(entries) <= upto and next_build[0] < n_heads:
                build_head(next_build[0])
                next_build[0] += 1

        # depth-2 software pipeline over the GLOBAL stream: keep two QK
        # slabs in flight ahead of PV so the exp engines run back-to-back
        # and the in-order PE never waits on them, even across heads.
        AHEAD = 2
        ensure_built(AHEAD)
        for i in range(min(AHEAD, len(entries))):
            entries[i][0]()
        i = 0
        while i < len(entries):
            ensure_built(i + AHEAD)
            if i + AHEAD < len(entries):
                flush_stores()
                entries[i + AHEAD][0]()
            entries[i][1]()
            i += 1

        flush_stores(force=True)

    nc.compile()
    return nc


def _get_program():
    key = "full"
    if key not in _build_cache:
        _build_cache[key] = build_program()
    return _build_cache[key]


def kernel(q, k, v):
    from concourse import bass_utils

    q = np.ascontiguousarray(np.asarray(q, dtype=np.float32))
    k = np.ascontiguousarray(np.asarray(k, dtype=np.float32))
    v = np.ascontiguousarray(np.asarray(v, dtype=np.float32))
    assert q.shape == (_B, _S, _H, _D), q.shape
    assert k.shape == (_B, _S, _HKV, _D), k.shape

    nc = _get_program()

    in_maps = []
    for c in range(_NCORES):
        b, p = divmod(c, _SHARDS)
        in_maps.append(
            {
                "q": np.ascontiguousarray(q[b, :, p * _H_PER : (p + 1) * _H_PER, :]),
                "k": np.ascontiguousarray(k[b, :, p * _KV_PER : (p + 1) * _KV_PER, :]),
                "v": np.ascontiguousarray(v[b, :, p * _KV_PER : (p + 1) * _KV_PER, :]),
            }
        )

    res = bass_utils.run_bass_kernel_spmd(nc, in_maps, list(range(_NCORES))).results

    out = np.empty((_B, _S, _H, _D), dtype=np.float32)
    for c in range(_NCORES):
        b, p = divmod(c, _SHARDS)
        out[b, :, p * _H_PER : (p + 1) * _H_PER, :] = res[c]["out"]
    return out


# revision 44
# speedup vs baseline: 1.0766x; 1.0766x over previous
"""Grouped-query causal attention on 8 TRN2 NeuronCores.

Problem: q [B=2, S=2048, H=32, D=128], k/v [B=2, S=2048, HKV=8, D=128],
causal softmax(q k^T / sqrt(D)) v with G = H // HKV = 4 query heads per
kv head.

Sharding (no collectives needed): 8 cores = 2 batches x 4 kv-head-pairs.
Each core computes 8 query heads / 2 kv heads of one batch element.

Per-core kernel design:
  - scores are built TRANSPOSED (S^T[k, q] tiles, k on partitions) so that
    softmax(P^T) feeds the P@V matmul directly as lhsT with no on-chip
    transposes at all.
  - Q^T/K^T [d, s] layouts come from a bf16 DRAM bounce (gpsimd casting
    DMA fp32->bf16) followed by a 2-byte xbar DMA transpose load.
  - causality is exploited at the slab level: per (q-block, k-tile-group)
    score slabs of [128, 1536] (3 PSUM banks, KG=3 k-tiles) for the fully
    causal region, plus ONE packed diagonal slab per q-block holding only
    the causal-needed columns of the 4 diagonal k-tiles (512+384+256+128 =
    1280 of 2048 cols; segments are packed to never cross a PSUM bank).
    This cuts both QK matmul columns and exp work by ~15% vs full-width.
  - exp is split across two engines to break the ScalarE bottleneck:
      * full slabs + qb0's diagonal slab: ScalarE ACTIVATE exp (exact),
        [128, up-to-1536] per instruction (amortizes the ~350cyc/instr
        overhead), bf16 out straight to SBUF.
      * diagonal slabs of qb1..3: VectorE via a Schraudolph bf16 exp:
        ONE fused scalar_tensor_tensor op computes
          int16 out = round(score * (128*scale/ln2) + maskbias)
        where maskbias = 16250.5 normally and +1200 on the masked upper
        triangle of diagonal blocks; the int16 bit pattern reinterpreted
        as bf16 IS 2^(t) with a linear-mantissa approx (~2% rel err on
        ~22% of elements -> ~0.5% output rel err), and masked entries
        become 2^-109 ~= 0. exp+causal-mask in one DVE op, no ScalarE.
  - softmax denominators ride along the P@V matmul as a ones-column
    appended to V (output column 128 = row sums), so no reductions are
    needed anywhere; finalize is a batched strided reciprocal + ONE
    broadcast tensor_tensor multiply per acc tile (2 q-tiles at a time).
  - PSUM budget: 2 score slabs (3 banks each) double-buffered + 2 acc
    tiles (1 bank each, 2 accumulator regions packed per bank) = 8 banks.
"""

import numpy as np

_B, _S, _H, _HKV, _D = 2, 2048, 32, 8, 128
_G = _H // _HKV  # 4 query heads per kv head
_NCORES = 8
_SHARDS = 4  # head shards; cores = _B * _SHARDS
_H_PER = _H // _SHARDS  # 8
_KV_PER = _HKV // _SHARDS  # 2

_P = 128  # partition / tile edge
_QB = 512  # q columns per block (4 q tiles)
_KG = 3  # k tiles per full PSUM score slab (3 banks)

# packed diagonal slab layout: local k-tile l -> (col offset, width)
# widths 512,384,256,128; 384+128 share a bank (384@512, 128@896) so no
# matmul output ever crosses a PSUM bank boundary.
_DIAG_OFF = {0: 0, 1: 512, 2: 1024, 3: 896}
_DIAG_W = {0: 512, 1: 384, 2: 256, 3: 128}
_DIAG_TOTAL = 1280

# Schraudolph bf16 exp constants: int16 = round(score*_SCHR_A + _SCHR_B)
_SCHR_A = float(_P / np.log(2.0) * (_D ** -0.5))  # fold 1/sqrt(D) scaling in
_SCHR_B = float(16256.0 - 5.5)
_SCHR_MASK = 1200.0  # masked entries -> bits ~<2700 -> bf16 ~2^-106 ~= 0

_build_cache = {}


def build_program(S=_S, n_heads=_H_PER, n_kv=_KV_PER, g=_G):
    """Emit + compile the single-core Tile program (SPMD: same NEFF on all
    cores, only the input data differs)."""
    import concourse.mybir as mybir
    import concourse.tile as tile
    from concourse import bacc
    from concourse.tile import add_dep_helper
    from contextlib import ExitStack

    dt = mybir.dt
    AF = mybir.ActivationFunctionType
    ALU = mybir.AluOpType

    D, P, QB, KG = _D, _P, _QB, _KG
    n_qt = S // P  # 128-row tiles along the sequence
    n_qb = S // QB  # q blocks
    qtb = QB // P  # q tiles per block (4)
    scale = float(D) ** -0.5

    nc = bacc.Bacc("TRN2", target_bir_lowering=False, debug=False)
    q_in = nc.dram_tensor("q", [S, n_heads, D], dt.float32, kind="ExternalInput").ap()
    k_in = nc.dram_tensor("k", [S, n_kv, D], dt.float32, kind="ExternalInput").ap()
    v_in = nc.dram_tensor("v", [S, n_kv, D], dt.float32, kind="ExternalInput").ap()
    o_out = nc.dram_tensor("out", [S, n_heads, D], dt.float32, kind="ExternalOutput").ap()

    with tile.TileContext(nc) as tc, ExitStack() as ctx:
        const_pool = ctx.enter_context(tc.tile_pool(name="const", bufs=1))
        dram_pool = ctx.enter_context(tc.tile_pool(name="bounce", bufs=12, space="DRAM"))
        qt_pool = ctx.enter_context(tc.tile_pool(name="qT", bufs=6))
        kt_pool = ctx.enter_context(tc.tile_pool(name="kT", bufs=2))
        v_pool = ctx.enter_context(tc.tile_pool(name="vv", bufs=2))
        fstg_pool = ctx.enter_context(tc.tile_pool(name="fstg", bufs=4))
        pt_pool = ctx.enter_context(tc.tile_pool(name="pT", bufs=4))
        osb_pool = ctx.enter_context(tc.tile_pool(name="osb", bufs=4))
        rc_pool = ctx.enter_context(tc.tile_pool(name="rc", bufs=8))
        sc_pool = ctx.enter_context(tc.tile_pool(name="sc", bufs=2, space="PSUM"))
        acc_pool = ctx.enter_context(tc.tile_pool(name="acc", bufs=2, space="PSUM"))

        def make_consts():
            # 0/1 lower-triangle (keep k<=q) for masking qb0's diagonal
            # blocks after its exact ScalarE exp.
            tri01 = const_pool.tile([P, P], dt.bfloat16)
            nc.gpsimd.memset(tri01[:], 1.0)
            nc.gpsimd.affine_select(
                out=tri01[:],
                in_=tri01[:],
                pattern=[[1, P]],
                base=0,
                channel_multiplier=-1,
                compare_op=ALU.is_ge,
                fill=0.0,
            )
            # Schraudolph bias-with-mask for the packed diagonal slab: the
            # value _SCHR_B everywhere, except the strict upper triangle
            # (k>q) of each segment's leading diagonal block, where a small
            # bias maps exp->~0.
            maskbias = const_pool.tile([P, _DIAG_TOTAL], dt.float32)
            nc.gpsimd.memset(maskbias[:], _SCHR_B)
            for l in range(qtb):
                off = _DIAG_OFF[l]
                nc.gpsimd.affine_select(
                    out=maskbias[:, off : off + P],
                    in_=maskbias[:, off : off + P],
                    pattern=[[1, P]],
                    base=0,
                    channel_multiplier=-1,
                    compare_op=ALU.is_ge,
                    fill=_SCHR_MASK,
                )
            return tri01, maskbias

        def load_xT_swdge(src, pool, tag, chunks=1):
            """SWDGE casting DMA(s) into a bf16 bounce + xbar load(s):
            slow (~40GB/s) but fully parallel to the HWDGE queues --
            carries the well-prefetched back-half tensors."""
            bounce = dram_pool.tile([S, D], dt.bfloat16, tag="bounce", name="bounce")
            xT = pool.tile([P, S], dt.bfloat16, tag=tag, name=tag)
            rows = S // chunks
            for c in range(chunks):
                sl = slice(c * rows, (c + 1) * rows)
                nc.gpsimd.dma_start(out=bounce[sl, :], in_=src[sl, :])
                nc.sync.dma_start_transpose(xT[:, sl], bounce[sl, :])
            return xT

        def cast_chunk_xT(src, xT, c, rows, eng, dq):
            """One chunk of the compute-cast transpose-load chain:
            fstg load -> compute cast (DVE 2x / gpsimd) -> bounce write ->
            xbar transpose load. All DMA hops of one chain ride ONE HWDGE
            queue (dq); different tensors' chains on different queues
            progress in parallel despite FIFO head-of-line blocking."""
            rt = rows // P
            sl = src[c * rows : (c + 1) * rows, :]
            stg = fstg_pool.tile([P, rt, D], dt.float32, tag="fstg", name="fstg")
            dq.dma_start(out=stg[:], in_=sl.rearrange("(t p) d -> p t d", p=P))
            stgb = fstg_pool.tile([P, rt, D], dt.bfloat16, tag="fstgb", name="fstgb")
            eng.tensor_copy(out=stgb[:], in_=stg[:])
            bounce = dram_pool.tile([rows, D], dt.bfloat16, tag="bounce", name="bounce")
            dq.dma_start(out=bounce[:].rearrange("(t p) d -> p t d", p=P), in_=stgb[:])
            dq.dma_start_transpose(xT[:, c * rows : (c + 1) * rows], bounce[:])

        def load_xT_cast(src, pool, tag, eng, chunks=1, dq=None):
            xT = pool.tile([P, S], dt.bfloat16, tag=tag, name=tag)
            rows = S // chunks
            for c in range(chunks):
                cast_chunk_xT(src, xT, c, rows, eng, dq or nc.sync)
            return xT

        def load_k(kv):
            if kv == 0:
                return load_xT_cast(k_in[:, kv, :], kt_pool, "kT", nc.vector, chunks=4)
            return load_xT_swdge(k_in[:, kv, :], kt_pool, "kT", chunks=2)

        def load_q(h):
            if h == 1:
                # startup-critical: rides the Scalar HWDGE queue, parallel
                # to head 0's chains on Sync
                return load_xT_cast(q_in[:, h, :], qt_pool, "qT", nc.vector,
                                    chunks=2, dq=nc.scalar)
            if h == 2:
                return load_xT_cast(q_in[:, h, :], qt_pool, "qT", nc.vector, chunks=2)
            if h == 3:
                return load_xT_cast(q_in[:, h, :], qt_pool, "qT", nc.vector, chunks=2)
            return load_xT_swdge(q_in[:, h, :], qt_pool, "qT")

        def load_v_chunk(vv, kv, c, rt, eng):
            vstg = fstg_pool.tile([P, rt, D], dt.float32, tag="fstg", name="fstg")
            nc.sync.dma_start(
                out=vstg[:],
                in_=v_in[c * rt * P : (c + 1) * rt * P, kv, :].rearrange(
                    "(t p) d -> p t d", p=P
                ),
            )
            eng.tensor_copy(out=vv[:, c * rt : (c + 1) * rt, 0:D], in_=vstg[:])

        def load_v(kv):
            # fp32 strided load -> compute cast into the vv tile. Row pitch
            # D+2 keeps bf16 rows 4B-aligned so the DVE cast runs in 2x mode.
            vv = v_pool.tile([P, n_qt, D + 2], dt.bfloat16, tag="vv", name="vv")
            nc.gpsimd.memset(vv[:, :, D], 1.0)
            eng = nc.vector if kv == 0 else nc.gpsimd
            chunks = 4 if kv == 0 else 2
            rt = n_qt // chunks
            for c in range(chunks):
                load_v_chunk(vv, kv, c, rt, eng)
            return vv

        # prefetched tiles, keyed by head / kv-head index
        kTs, qTs, vvs = {}, {}, {}

        def prefetch(hh):
            if hh >= n_heads:
                return
            hkv = hh // g
            if hkv not in kTs:
                kTs[hkv] = load_k(hkv)
            if hh not in qTs:
                qTs[hh] = load_q(hh)
            if hkv not in vvs:
                vvs[hkv] = load_v(hkv)

        def prefetch0():
            """Head 0's loads: k chain on the Sync HWDGE queue and q chain
            on the Scalar HWDGE queue IN PARALLEL (each chain is internally
            FIFO-serialized; two queues let k c0 and q c0 both land within
            ~10us). v rides Sync fstg + DVE cast (no transpose needed)."""
            kT = kt_pool.tile([P, S], dt.bfloat16, tag="kT", name="kT")
            qT = qt_pool.tile([P, S], dt.bfloat16, tag="qT", name="qT")
            vv = v_pool.tile([P, n_qt, D + 2], dt.bfloat16, tag="vv", name="vv")
            nc.gpsimd.memset(vv[:, :, D], 1.0)
            rows = S // 4
            for c in range(4):
                cast_chunk_xT(k_in[:, 0, :], kT, c, rows, nc.vector, nc.sync)
                cast_chunk_xT(q_in[:, 0, :], qT, c, rows, nc.vector, nc.scalar)
                load_v_chunk(vv, 0, c, rows // P, nc.vector)
            kTs[0], qTs[0], vvs[0] = kT, qT, vv

        prefetch0()
        prefetch(1)
        tri01, maskbias = make_consts()
        prefetch(2)

        # Out-stores are deferred ~2 slabs so that, by the time the Sync
        # FIFO queue head reaches a store trigger, its finalize data is
        # already computed -- a store that waits at queue head blocks every
        # load DMA queued behind it.
        pending_stores = []

        def flush_stores(force=False):
            while pending_stores and (force or len(pending_stores) > 2):
                o_sb_p, qb_p, h_p = pending_stores.pop(0)
                nc.sync.dma_start(
                    out=o_out[qb_p * QB : (qb_p + 1) * QB, h_p, :].rearrange(
                        "(t p) d -> p t d", p=P
                    ),
                    in_=o_sb_p[:],
                )

        # ------- global cross-head slab stream -------
        # The PE is in-order: if head h+1's first QK were only emitted
        # after head h's last PV, the PE would idle for the final diag
        # slab's DVE-exp latency at every head boundary. Building one
        # global slab stream lets the QK lookahead cross head boundaries.
        entries = []  # (qk_thunk, pv_thunk) per slab, all heads
        next_build = [0]

        def build_head(h):
            kv = h // g
            prefetch(h + 3)  # keep load chains ~3 head-windows ahead
            kT = kTs[kv]
            qT = qTs.pop(h)
            vv = vvs[kv]
            if h % g == g - 1:  # last head using this kv group
                del kTs[kv], vvs[kv]

            # slab list for this head:
            #   ("full", qb, j0, j1): k tiles [j0, j1) at full 512 width
            #   ("diag", qb): the 4 diagonal k-tiles, packed causal widths
            slabs = []
            for qb in range(n_qb):
                nfull = qb * qtb
                for j0 in range(0, nfull, KG):
                    slabs.append(("full", qb, j0, min(j0 + KG, nfull)))
                slabs.append(("diag", qb, 0, 0))

            accs_of = {}  # qb -> 2 accumulator tiles (2 regions each)
            live = {}  # slab idx -> pT tile

            def emit_qk(si, kT=kT, qT=qT, h=h, slabs=slabs, live=live):
                kind, qb, j0, j1 = slabs[si]
                sc = sc_pool.tile([P, KG * QB], dt.float32, tag="sc", name="sc")
                pT = pt_pool.tile([P, KG * QB], dt.bfloat16, tag="pT", name="pT")
                if kind == "full":
                    for j in range(j0, j1):
                        jl = j - j0
                        nc.tensor.matmul(
                            out=sc[:, jl * QB : (jl + 1) * QB],
                            lhsT=kT[:, j * P : (j + 1) * P],
                            rhs=qT[:, qb * QB : (qb + 1) * QB],
                            start=True,
                            stop=True,
                        )
                    W = (j1 - j0) * QB
                    nc.scalar.activation(
                        out=pT[:, :W], in_=sc[:, :W], func=AF.Exp, scale=scale
                    )
                else:
                    for l in range(qtb):
                        j = qb * qtb + l
                        off, w = _DIAG_OFF[l], _DIAG_W[l]
                        nc.tensor.matmul(
                            out=sc[:, off : off + w],
                            lhsT=kT[:, j * P : (j + 1) * P],
                            rhs=qT[:, qb * QB + l * P : (qb + 1) * QB],
                            start=True,
                            stop=True,
                        )
                    if qb == 0:
                        # exact exp; mask diag blocks after (VectorE, cheap)
                        nc.scalar.activation(
                            out=pT[:, :_DIAG_TOTAL],
                            in_=sc[:, :_DIAG_TOTAL],
                            func=AF.Exp,
                            scale=scale,
                        )
                        for l in range(qtb):
                            off = _DIAG_OFF[l]
                            blk = pT[:, off : off + P]
                            nc.vector.tensor_tensor(
                                out=blk, in0=blk, in1=tri01[:], op=ALU.mult
                            )
                    else:
                        # Schraudolph bf16 exp + causal mask, ONE DVE op:
                        # int16 = round(score * A + maskbias)
                        nc.vector.scalar_tensor_tensor(
                            out=pT[:, :_DIAG_TOTAL].bitcast(dt.int16),
                            in0=sc[:, :_DIAG_TOTAL],
                            scalar=_SCHR_A,
                            in1=maskbias[:],
                            op0=ALU.mult,
                            op1=ALU.add,
                        )
                live[si] = pT

            def emit_pv(si, vv=vv, h=h, slabs=slabs, live=live, accs_of=accs_of):
                kind, qb, j0, j1 = slabs[si]
                pT = live.pop(si)
                if (kind == "full" and j0 == 0) or (kind == "diag" and qb == 0):
                    # two accumulators packed per PSUM bank; region r of a
                    # tile is cols [r*(D+1), (r+1)*(D+1)). Only region 0's
                    # first matmul uses start=True (clears the whole bank's
                    # has_written bits); region 1's first matmul relies on
                    # still-pending bits to overwrite, so it must execute
                    # after region 0's start (manual dep below).
                    accs_of[qb] = [
                        acc_pool.tile([P, 2 * (D + 1)], dt.float32, tag="acc", name=f"accp{r}")
                        for r in range(qtb // 2)
                    ]
                accs = accs_of[qb]
                first_mm = {}

                def pv_mm(lhs_ap, it, first, last):
                    tile_, r = accs[it // 2], it % 2
                    mm = nc.tensor.matmul(
                        out=tile_[:, r * (D + 1) : (r + 1) * (D + 1)],
                        lhsT=lhs_ap,
                        rhs=vv[:, pv_mm.j, 0 : D + 1],
                        start=(first and r == 0),
                        stop=last,
                        skip_group_check=True,
                    )
                    if first:
                        first_mm[it] = mm
                        if r == 1:
                            add_dep_helper(
                                mm.ins,
                                first_mm[it - 1].ins,
                                sync=False,
                                reason="acc bank-mate ordering (pending-zero)",
                            )

                if kind == "full":
                    for j in range(j0, j1):
                        jl = j - j0
                        pv_mm.j = j
                        for it in range(qtb):
                            qt_abs = qb * qtb + it
                            pv_mm(
                                pT[:, jl * QB + it * P : jl * QB + (it + 1) * P],
                                it,
                                first=(j == 0),
                                last=False,
                            )
                else:
                    for l in range(qtb):
                        j = qb * qtb + l
                        off = _DIAG_OFF[l]
                        pv_mm.j = j
                        for it in range(l, qtb):
                            pv_mm(
                                pT[:, off + (it - l) * P : off + (it - l + 1) * P],
                                it,
                                first=(qb == 0 and l == 0),
                                last=(l == it),
                            )
                    # finalize this q block: batched reciprocal of the two
                    # ride-along denominators per acc tile, then ONE
                    # broadcast multiply per acc tile (2 q-tiles at once).
                    o_sb = osb_pool.tile([P, qtb, D], dt.float32, tag="osb", name="osb")
                    for t in range(qtb // 2):
                        tile_ = accs[t]
                        rc = rc_pool.tile([P, 2], dt.float32, tag="rc", name="rc")
                        nc.vector.reciprocal(rc[:], tile_[:, D :: D + 1])
                        nc.vector.tensor_tensor(
                            out=o_sb[:, 2 * t : 2 * t + 2, :],
                            in0=tile_[:].rearrange("p (t c) -> p t c", t=2)[:, :, 0:D],
                            in1=rc[:].unsqueeze(2).broadcast_to([P, 2, D]),
                            op=ALU.mult,
                        )
                    pending_stores.append((o_sb, qb, h))
                    del accs_of[qb]

            for si in range(len(slabs)):
                entries.append((lambda si=si, f=emit_qk: f(si),
                                lambda si=si, f=emit_pv: f(si)))

        def ensure_built(upto):
            while len(entries) <= upto and next_build[0] < n_heads:
                build_head(next_build[0])
                next_build[0] += 1

        # depth-2 software pipeline over the GLOBAL stream: keep two QK
        # slabs in flight ahead of PV so the exp engines run back-to-back
        # and the in-order PE never waits on them, even across heads.
        AHEAD = 2
        ensure_built(AHEAD)
        for i in range(min(AHEAD, len(entries))):
            entries[i][0]()
        i = 0
        while i < len(entries):
            ensure_built(i + AHEAD)
            if i + AHEAD < len(entries):
                flush_stores()
                entries[i + AHEAD][0]()
            entries[i][1]()
            i += 1

        flush_stores(force=True)

    nc.compile()
    return nc


def _get_program():
    key = "full"
    if key not in _build_cache:
        _build_cache[key] = build_program()
    return _build_cache[key]


def kernel(q, k, v):
    from concourse import bass_utils

    q = np.ascontiguousarray(np.asarray(q, dtype=np.float32))
    k = np.ascontiguousarray(np.asarray(k, dtype=np.float32))
    v = np.ascontiguousarray(np.asarray(v, dtype=np.float32))
    assert q.shape == (_B, _S, _H, _D), q.shape
    assert k.shape == (_B, _S, _HKV, _D), k.shape

    nc = _get_program()

    in_maps = []
    for c in range(_NCORES):
        b, p = divmod(c, _SHARDS)
        in_maps.append(
            {
                "q": np.ascontiguousarray(q[b, :, p * _H_PER : (p + 1) * _H_PER, :]),
                "k": np.ascontiguousarray(k[b, :, p * _KV_PER : (p + 1) * _KV_PER, :]),
                "v": np.ascontiguousarray(v[b, :, p * _KV_PER : (p + 1) * _KV_PER, :]),
            }
        )

    res = bass_utils.run_bass_kernel_spmd(nc, in_maps, list(range(_NCORES))).results

    out = np.empty((_B, _S, _H, _D), dtype=np.float32)
    for c in range(_NCORES):
        b, p = divmod(c, _SHARDS)
        out[b, :, p * _H_PER : (p + 1) * _H_PER, :] = res[c]["out"]
    return out
